# revision 8
# baseline (speedup 1.0000x reference)
"""GA3 Conv2d kernel for 8 Trainium2 NeuronCores.

Math: the reference computes, per batch image,
    out[b, co, m] = sum_{j,k} S[m,j,k] * (conv2d(a_k, W[j]) + bias[j])[co]
with a_k = x[:, k::8] (blade-interleaved channels).  Because the sign
combination is linear, it folds into the conv weights:
    V[co*8+m, ci*8+k, kh, kw] = sum_j S[m,j,k] * W[j, co, ci, kh, kw]
    bias_eff[co*8+m]          = sum_{j,k} S[m,j,k] * b[j, co]
so the whole module is ONE dense 3x3 conv with Cin=Cout=128 on
[B, 128, 128, 128].  We shard data-parallel over B across the 8 cores
(1 image per core) and implement the conv as 9 shifted fp32r matmuls
(tap weights stationary [ic=128 x oc=128], pixels moving) accumulated
in PSUM.
"""

import numpy as np

_TERMS = [
    [(0, 0, 1), (1, 1, 1), (2, 2, 1), (3, 3, 1), (4, 4, -1), (5, 5, -1), (6, 6, -1), (7, 7, -1)],
    [(1, 0, 1), (0, 1, 1), (2, 4, 1), (4, 2, -1), (3, 6, 1), (6, 3, -1), (5, 7, -1), (7, 5, -1)],
    [(2, 0, 1), (0, 2, 1), (1, 4, -1), (4, 1, 1), (3, 5, 1), (5, 3, -1), (6, 7, 1), (7, 6, 1)],
    [(3, 0, 1), (0, 3, 1), (1, 6, -1), (6, 1, 1), (2, 5, -1), (5, 2, 1), (4, 7, -1), (7, 4, -1)],
    [(4, 0, 1), (0, 4, 1), (2, 1, 1), (1, 2, -1), (3, 7, 1), (7, 3, 1), (6, 5, 1), (5, 6, -1)],
    [(5, 0, 1), (0, 5, 1), (3, 2, 1), (2, 3, -1), (1, 7, 1), (7, 1, 1), (4, 6, 1), (6, 4, -1)],
    [(6, 0, 1), (0, 6, 1), (3, 1, 1), (1, 3, -1), (2, 7, -1), (7, 2, -1), (5, 4, 1), (4, 5, -1)],
    [(7, 0, 1), (0, 7, 1), (5, 1, 1), (1, 5, 1), (6, 2, -1), (2, 6, -1), (4, 3, 1), (3, 4, 1)],
]
_S = np.zeros((8, 8, 8), dtype=np.float32)
for _m, _terms in enumerate(_TERMS):
    for _j, _k, _s in _terms:
        _S[_m, _j, _k] = _s

B, CIN, COUT, H, W = 8, 16, 16, 128, 128
C = 8 * CIN  # 128 interleaved channels
N_CORES = 8
STRIP = 16          # output rows per strip
N_STRIPS = H // STRIP
GROUP = 4           # output rows per PSUM accumulation group (4*128 = 512 free)
PW = W + 2          # padded width

_CACHED_NC = None


def _build_nc():
    import concourse.mybir as mybir
    import concourse.tile as tile
    from concourse import bacc

    f32 = mybir.dt.float32
    f32r = mybir.dt.float32r

    nc = bacc.Bacc("TRN2", target_bir_lowering=False, debug=False,
                   enable_asserts=False)

    xb = nc.dram_tensor("xb", [C, H, W], f32r, kind="ExternalInput").ap()
    wt = nc.dram_tensor("wt", [C, 9, C], f32r, kind="ExternalInput").ap()
    bi = nc.dram_tensor("bi", [C, 1], f32, kind="ExternalInput").ap()
    out = nc.dram_tensor("out", [C, H, W], f32, kind="ExternalOutput").ap()
    # Zero source for the padding halo: ExternalOutputs are pre-zeroed by
    # the runner (native path zero-fills, PJRT path donates zero buffers),
    # and memset cannot target float32r SBUF — so pads are DMAd from here.
    zsrc = nc.dram_tensor("zsrc", [C, PW], f32r, kind="ExternalOutput").ap()

    with tile.TileContext(nc) as tc:
        with (
            tc.tile_pool(name="wpool", bufs=1) as wpool,
            tc.tile_pool(name="xpool", bufs=3) as xpool,
            tc.tile_pool(name="pspool", bufs=8, space="PSUM") as pspool,
            tc.tile_pool(name="opool", bufs=3) as opool,
        ):
            wtile = wpool.tile([C, 9, C], f32r)
            nc.sync.dma_start(out=wtile[:, :, :], in_=wt[:, :, :])
            btile = wpool.tile([C, 1], f32)
            nc.sync.dma_start(out=btile[:, :], in_=bi[:, :])

            for s in range(N_STRIPS):
                # xt holds input rows [16s-1, 16s+17) with one zero column on
                # each side; local row lr maps to global row 16s-1+lr.
                xt = xpool.tile([C, STRIP + 2, PW], f32r)
                nc.sync.dma_start(out=xt[:, :, 0:1], in_=zsrc[:, 0:STRIP + 2])
                nc.sync.dma_start(out=xt[:, :, PW - 1:PW],
                                  in_=zsrc[:, 0:STRIP + 2])
                if s == 0:
                    nc.sync.dma_start(out=xt[:, 0:1, 1:PW - 1],
                                      in_=zsrc[:, 0:W])
                    nc.sync.dma_start(out=xt[:, 1:STRIP + 2, 1:PW - 1],
                                      in_=xb[:, 0:STRIP + 1, :])
                elif s == N_STRIPS - 1:
                    nc.sync.dma_start(out=xt[:, STRIP + 1:STRIP + 2, 1:PW - 1],
                                      in_=zsrc[:, 0:W])
                    nc.sync.dma_start(out=xt[:, 0:STRIP + 1, 1:PW - 1],
                                      in_=xb[:, s * STRIP - 1:H, :])
                else:
                    nc.sync.dma_start(out=xt[:, :, 1:PW - 1],
                                      in_=xb[:, s * STRIP - 1:s * STRIP + STRIP + 1, :])

                obuf = opool.tile([C, STRIP, W], f32)
                for g in range(STRIP // GROUP):
                    ps = pspool.tile([C, GROUP, W], f32)
                    ta = 0
                    for dh in range(3):
                        for dw in range(3):
                            rhs = xt[:, 4 * g + dh:4 * g + dh + GROUP, dw:dw + W]
                            nc.tensor.matmul(
                                ps[:, :, :],
                                lhsT=wtile[:, ta, :],
                                rhs=rhs,
                                start=(ta == 0),
                                stop=(ta == 8),
                            )
                            ta += 1
                    nc.vector.tensor_scalar_add(
                        out=obuf[:, 4 * g:4 * g + GROUP, :],
                        in0=ps[:, :, :],
                        scalar1=btile[:, 0:1],
                    )
                nc.sync.dma_start(out=out[:, s * STRIP:(s + 1) * STRIP, :],
                                  in_=obuf[:, :, :])

    nc.compile()
    return nc


def _get_nc():
    global _CACHED_NC
    if _CACHED_NC is None:
        _CACHED_NC = _build_nc()
    return _CACHED_NC


def _prep_weights(Wfull: np.ndarray, b: np.ndarray):
    # wt[ic, tap, oc] with ic = ci*8+k, oc = co*8+m, tap = kh*3+kw
    V = np.einsum("mjk,jcihw->ikhwcm", _S.astype(np.float64),
                  Wfull.astype(np.float64))          # [ci,k,kh,kw,co,m]
    V = V.reshape(C, 3, 3, C).reshape(C, 9, C)
    bias = np.einsum("mjk,jc->cm", _S.astype(np.float64),
                     b.astype(np.float64)).reshape(C, 1)
    return np.ascontiguousarray(V, dtype=np.float32), \
        np.ascontiguousarray(bias, dtype=np.float32)


def kernel(x: np.ndarray, W: np.ndarray, b: np.ndarray) -> np.ndarray:
    from concourse.bass_utils import run_bass_kernel_spmd

    x = np.ascontiguousarray(x, dtype=np.float32)
    wt, bias = _prep_weights(np.asarray(W), np.asarray(b))

    nc = _get_nc()
    in_maps = [{"xb": x[c], "wt": wt, "bi": bias} for c in range(N_CORES)]
    res = run_bass_kernel_spmd(nc, in_maps, core_ids=list(range(N_CORES)))
    return np.stack([res.results[c]["out"] for c in range(N_CORES)], axis=0)


# revision 15
# speedup vs baseline: 1.0377x; 1.0377x over previous
"""GA3 Conv2d kernel for 8 Trainium2 NeuronCores.

Math: the reference computes, per batch image,
    out[b, co, m] = sum_{j,k} S[m,j,k] * (conv2d(a_k, W[j]) + bias[j])[co]
with a_k = x[:, k::8] (blade-interleaved channels).  Because the sign
combination is linear, it folds into the conv weights:
    V[co*8+m, ci*8+k, kh, kw] = sum_j S[m,j,k] * W[j, co, ci, kh, kw]
    bias_eff[co*8+m]          = sum_{j,k} S[m,j,k] * b[j, co]
so the whole module is ONE dense 3x3 conv with Cin=Cout=128 on
[B, 128, 128, 128].  We shard data-parallel over B across the 8 cores
(1 image per core) and implement the conv as 9 shifted fp32r matmuls
(tap weights stationary [ic=128 x oc=128], pixels moving) accumulated
in PSUM.
"""

import numpy as np

_TERMS = [
    [(0, 0, 1), (1, 1, 1), (2, 2, 1), (3, 3, 1), (4, 4, -1), (5, 5, -1), (6, 6, -1), (7, 7, -1)],
    [(1, 0, 1), (0, 1, 1), (2, 4, 1), (4, 2, -1), (3, 6, 1), (6, 3, -1), (5, 7, -1), (7, 5, -1)],
    [(2, 0, 1), (0, 2, 1), (1, 4, -1), (4, 1, 1), (3, 5, 1), (5, 3, -1), (6, 7, 1), (7, 6, 1)],
    [(3, 0, 1), (0, 3, 1), (1, 6, -1), (6, 1, 1), (2, 5, -1), (5, 2, 1), (4, 7, -1), (7, 4, -1)],
    [(4, 0, 1), (0, 4, 1), (2, 1, 1), (1, 2, -1), (3, 7, 1), (7, 3, 1), (6, 5, 1), (5, 6, -1)],
    [(5, 0, 1), (0, 5, 1), (3, 2, 1), (2, 3, -1), (1, 7, 1), (7, 1, 1), (4, 6, 1), (6, 4, -1)],
    [(6, 0, 1), (0, 6, 1), (3, 1, 1), (1, 3, -1), (2, 7, -1), (7, 2, -1), (5, 4, 1), (4, 5, -1)],
    [(7, 0, 1), (0, 7, 1), (5, 1, 1), (1, 5, 1), (6, 2, -1), (2, 6, -1), (4, 3, 1), (3, 4, 1)],
]
_S = np.zeros((8, 8, 8), dtype=np.float32)
for _m, _terms in enumerate(_TERMS):
    for _j, _k, _s in _terms:
        _S[_m, _j, _k] = _s

B, CIN, COUT, H, W = 8, 16, 16, 128, 128
C = 8 * CIN  # 128 interleaved channels
N_CORES = 8
STRIP = 16          # output rows per strip
N_STRIPS = H // STRIP
GROUP = 4           # output rows per PSUM accumulation group (4*128 = 512 free)
PW = W + 2          # padded width

_CACHED_NC = None


def _build_nc():
    import concourse.bass as bass
    import concourse.mybir as mybir
    import concourse.tile as tile
    from concourse import bacc

    f32 = mybir.dt.float32
    f32r = mybir.dt.float32r

    nc = bacc.Bacc("TRN2", target_bir_lowering=False, debug=False,
                   enable_asserts=False)

    xb = nc.dram_tensor("xb", [C, H, W], f32r, kind="ExternalInput").ap()
    wt = nc.dram_tensor("wt", [C, 9, C], f32r, kind="ExternalInput").ap()
    bi = nc.dram_tensor("bi", [C, 1], f32, kind="ExternalInput").ap()
    out = nc.dram_tensor("out", [C, H, W], f32, kind="ExternalOutput").ap()
    # Zero source for the padding halo: ExternalOutputs are pre-zeroed by
    # the runner (native path zero-fills, PJRT path donates zero buffers),
    # and memset cannot target float32r SBUF — so pads are DMAd from here.
    zsrc = nc.dram_tensor("zsrc", [C, PW], f32r, kind="ExternalOutput").ap()

    with tile.TileContext(nc) as tc:
        with (
            tc.tile_pool(name="wpool", bufs=1) as wpool,
            tc.tile_pool(name="xpool", bufs=3) as xpool,
            tc.tile_pool(name="pspool", bufs=8, space="PSUM") as pspool,
            tc.tile_pool(name="opool", bufs=3) as opool,
        ):
            # Constants + pads + output stores ride the ACT HWDGE ring
            # (nc.scalar); strip input loads get the SP ring (nc.sync) to
            # themselves so the first matmul isn't stuck behind a serial
            # DMA chain and mid-kernel loads don't compete with stores.
            wtile = wpool.tile([C, 9, C], f32r)
            nc.scalar.dma_start(out=wtile[:, :, :], in_=wt[:, :, :])
            btile = wpool.tile([C, 1], f32)
            nc.scalar.dma_start(out=btile[:, :], in_=bi[:, :])

            NR = STRIP + 2        # input rows held per strip (with halo)
            FLAT = 2 + NR * PW    # [pad pair][row: 128 data + pad pair]*NR
            for s in range(N_STRIPS):
                # xt flat layout: offsets {130k, 130k+1} are zero pads, row
                # lr's data lives at [2 + 130*lr, 2 + 130*lr + 128); local
                # row lr maps to global input row 16s-1+lr.  Every pad cell
                # lies on one [[130,19],[1,2]] run -> a single zero DMA.
                xt = xpool.tile([C, FLAT], f32r)
                pad_ap = bass.AP(xt.tensor, xt.offset,
                                 [xt.ap[0], [PW, NR + 1], [1, 2]])
                # stride-4 outer keeps the source from folding to 1-D so it
                # balances 1:1 against the strided destination
                zsrc_ap = bass.AP(zsrc.tensor, 0,
                                  [zsrc.ap[0], [4, NR + 1], [1, 2]])
                nc.scalar.dma_start(out=pad_ap, in_=zsrc_ap)

                def row_dst(lr, nrows):
                    return bass.AP(xt.tensor, xt.offset + 2 + PW * lr,
                                   [xt.ap[0], [PW, nrows], [1, W]])

                if s == 0:
                    nc.scalar.dma_start(out=row_dst(0, 1), in_=zsrc[:, 0:W])
                    # split so the first PSUM groups can start after ~half
                    # the strip has landed
                    nc.sync.dma_start(out=row_dst(1, 9), in_=xb[:, 0:9, :])
                    nc.sync.dma_start(out=row_dst(10, STRIP - 8),
                                      in_=xb[:, 9:STRIP + 1, :])
                elif s == N_STRIPS - 1:
                    nc.scalar.dma_start(out=row_dst(STRIP + 1, 1),
                                        in_=zsrc[:, 0:W])
                    nc.sync.dma_start(out=row_dst(0, STRIP + 1),
                                      in_=xb[:, s * STRIP - 1:H, :])
                else:
                    nc.sync.dma_start(out=row_dst(0, NR),
                                      in_=xb[:, s * STRIP - 1:s * STRIP + STRIP + 1, :])

                obuf = opool.tile([C, STRIP * W], f32)
                for g in range(STRIP // GROUP):
                    ps = pspool.tile([C, GROUP * W], f32)
                    ta = 0
                    for dh in range(3):
                        for dw in range(3):
                            rhs = bass.AP(
                                xt.tensor,
                                xt.offset + 1 + PW * (4 * g + dh) + dw,
                                [xt.ap[0], [PW, GROUP], [1, W]])
                            nc.tensor.matmul(
                                ps[:, :],
                                lhsT=wtile[:, ta, :],
                                rhs=rhs,
                                start=(ta == 0),
                                stop=(ta == 8),
                            )
                            ta += 1
                    nc.vector.tensor_scalar_add(
                        out=obuf[:, g * GROUP * W:(g + 1) * GROUP * W],
                        in0=ps[:, :],
                        scalar1=btile[:, 0:1],
                    )
                nc.scalar.dma_start(out=out[:, s * STRIP:(s + 1) * STRIP, :],
                                    in_=obuf[:, :])

    nc.compile()
    return nc


def _get_nc():
    global _CACHED_NC
    if _CACHED_NC is None:
        _CACHED_NC = _build_nc()
    return _CACHED_NC


def _prep_weights(Wfull: np.ndarray, b: np.ndarray):
    # wt[ic, tap, oc] with ic = ci*8+k, oc = co*8+m, tap = kh*3+kw
    V = np.einsum("mjk,jcihw->ikhwcm", _S.astype(np.float64),
                  Wfull.astype(np.float64))          # [ci,k,kh,kw,co,m]
    V = V.reshape(C, 3, 3, C).reshape(C, 9, C)
    bias = np.einsum("mjk,jc->cm", _S.astype(np.float64),
                     b.astype(np.float64)).reshape(C, 1)
    return np.ascontiguousarray(V, dtype=np.float32), \
        np.ascontiguousarray(bias, dtype=np.float32)


def kernel(x: np.ndarray, W: np.ndarray, b: np.ndarray) -> np.ndarray:
    from concourse.bass_utils import run_bass_kernel_spmd

    x = np.ascontiguousarray(x, dtype=np.float32)
    wt, bias = _prep_weights(np.asarray(W), np.asarray(b))

    nc = _get_nc()
    in_maps = [{"xb": x[c], "wt": wt, "bi": bias} for c in range(N_CORES)]
    res = run_bass_kernel_spmd(nc, in_maps, core_ids=list(range(N_CORES)))
    return np.stack([res.results[c]["out"] for c in range(N_CORES)], axis=0)
